# revision 21
# baseline (speedup 1.0000x reference)
"""Trainium2 Bass kernel for nn_CSB (dense_transformer).

Reference computation (per sample b of B=4, N=16384, C=384, d=192, H=W=128,
M=N/16=1024):
  x1 = x[..., :d]; x2 = x[..., d:]
  x2c  = conv4x4s4(x2 as [d,H,W]) + conv_b            # [d, M]
  gate = sigmoid(x1 @ x2c)                            # [N, M]
  sp   = gate @ x2c.T                                 # [N, d]
  att  = softmax(x1.T @ x2, axis over first d)        # [d, d]
  ch   = x2 @ att                                     # [N, d]
  cat  = [sp, ch]; ln = LN(cat) * ln_w + ln_b
  out  = (ln @ proj_w.T + proj_b).T                   # [C, N]

Sharding: 8 cores = 4 samples x 2 N-halves. Each core gets the FULL sample
(halves swapped for half-1 cores so every core "owns" rows 0:8192) and
produces out[b][:, half*8192:(half+1)*8192].

Layout strategy (v2):
  - Host supplies, per core: x in fp16 natural layout (feeds only the
    s-matrix; fp16 matmuls run 1 cyc/row at any free size so free=192 needs
    no fp32r padding), plus PRE-TRANSPOSED fp16 x2T (full N, for conv/ch)
    and x1T (owned half, for the gate) — no on-chip PE transposes of x and
    no x1 re-load in the main loop.
  - conv weights are host-prepped [d_in, 16, 193] fp16 where output col 192
    is sum_o w[..,o]; the conv's pc1 psum row 64 then directly yields
    colsum(x2c)[m], which rides along the x2cT transposes into an appended
    lhsT column -> sum_d spT arrives as row 64 of the sp1 psum for free.
  - att tiles carry rowsum(att) as col 0, so sum_e chT arrives as row 0 of
    the cha psum. mean(cat) is then one DVE row-add; no stats matmuls.
  - LN folded into the projection:
      out = (pwTs.T @ catT + pwsumneg x Sigma) * rstd_bc + bias2
    with pwTs = proj_w.T * ln_w, pwsumneg = -colsum(pwTs)/C,
    bias2 = proj_w @ ln_b + proj_b.
  - Sqrt ops batched 4 blocks at a time so the ACT engine swaps between the
    sigmoid and sqrt table sets twice per 4 blocks instead of per block;
    squares and bias-adds run on the otherwise-idle GPSIMD engine.
  - Output written fp16 (within tolerance), host upcasts to f32.
"""

import sys
import types

_m = types.ModuleType("antenv.axon_hooks")
_m.get_axon_ntff_profile_hook = lambda: None
sys.modules.setdefault("antenv.axon_hooks", _m)

import numpy as np

import concourse.bacc as bacc
import concourse.mybir as mybir
import concourse.tile as tile
from concourse.masks import make_identity

F32 = mybir.dt.float32
F32R = mybir.dt.float32r
F16 = mybir.dt.float16
AF = mybir.ActivationFunctionType
OP = mybir.AluOpType

B = 4
N = 16384
C = 384
D = 192  # C // 2
M = 1024  # N // 16
NH = 8192  # N // 2, rows per core
NBLK = 512  # n-columns per main-loop block
NBLOCKS = NH // NBLK  # 16
EPS = 1e-5


def build_nc():
    nc = bacc.Bacc(None, target_bir_lowering=False)

    x16 = nc.dram_tensor("x16", [N, C], F16, kind="ExternalInput")
    xt1 = nc.dram_tensor("xt1", [D, NH], F16, kind="ExternalInput")
    xt2 = nc.dram_tensor("xt2", [D, N], F16, kind="ExternalInput")
    convw = nc.dram_tensor("convw", [D, 16, 193], F16, kind="ExternalInput")
    convb = nc.dram_tensor("convb", [193], F32, kind="ExternalInput")
    lnw_d = nc.dram_tensor("lnw", [C], F32, kind="ExternalInput")
    lnb_d = nc.dram_tensor("lnb", [C], F32R, kind="ExternalInput")
    pwT_d = nc.dram_tensor("pwT", [C, C], F32R, kind="ExternalInput")
    pb_d = nc.dram_tensor("pb", [C], F32, kind="ExternalInput")
    out_part = nc.dram_tensor("out_part", [C, NH], F16, kind="ExternalOutput")

    with tile.TileContext(nc) as tc:
        import contextlib

        with contextlib.ExitStack() as top:
            const = top.enter_context(tc.tile_pool(name="const", bufs=1))
            big = top.enter_context(tc.tile_pool(name="big", bufs=1))

            # ---------------- constants ----------------
            ident_f = const.tile([128, 128], F32, tag="ident_f")
            make_identity(nc, ident_f[:])
            ident16 = const.tile([128, 128], F16, tag="ident16")
            nc.vector.tensor_copy(ident16[:], ident_f[:])

            ones_f = const.tile([128, 1], F32, tag="ones_f")
            nc.gpsimd.memset(ones_f[:], 1.0)
            invC_f = const.tile([128, 1], F32, tag="invC_f")
            nc.gpsimd.memset(invC_f[:], 1.0 / C)
            invC_col = const.tile([128, 1], F32R, tag="invC_col")
            nc.vector.tensor_copy(invC_col[:], invC_f[:])
            onesr_f = const.tile([1, 128], F32, tag="onesr_f")
            nc.gpsimd.memset(onesr_f[:], 1.0)
            onesr_r = const.tile([1, 128], F32R, tag="onesr_r")
            nc.vector.tensor_copy(onesr_r[:], onesr_f[:])
            eps_sb = const.tile([1, 1], F32, tag="eps_sb")
            nc.gpsimd.memset(eps_sb[:], EPS)

            # per-channel vectors as [128, k] column stacks
            lnw_sb = const.tile([128, 3], F32, tag="lnw_sb")
            nc.sync.dma_start(lnw_sb[:], lnw_d.ap().rearrange("(o p) -> p o", p=128))
            lnb_sb = const.tile([128, 3], F32R, tag="lnb_sb")
            nc.sync.dma_start(lnb_sb[:], lnb_d.ap().rearrange("(o p) -> p o", p=128))
            pb_sb = const.tile([128, 3], F32, tag="pb_sb")
            nc.sync.dma_start(pb_sb[:], pb_d.ap().rearrange("(o p) -> p o", p=128))
            convb_sb = const.tile([128, 2], F32, tag="convb_sb")
            nc.sync.dma_start(convb_sb[:, 0:1], convb.ap()[0:128, None])
            nc.sync.dma_start(convb_sb[0:65, 1:2], convb.ap()[128:193, None])

            # proj weights: pwT [c, o]; pwTs = pwT * ln_w[c];
            # bias2 = P@lnb + pb; pwsumneg = -colsum(pwTs)/C
            pwTs = [
                const.tile([128, C], F32R, tag=f"pwTs{i}", name=f"pwTs{i}")
                for i in range(3)
            ]
            with tc.tile_pool(name="pwload", bufs=1) as pwload, \
                 tc.tile_pool(name="pwpsum", bufs=1, space="PSUM") as pwpsum:
                pwt_raw = [
                    pwload.tile([128, C], F32R, tag=f"pwt{i}", name=f"pwt{i}")
                    for i in range(3)
                ]
                for i in range(3):
                    nc.sync.dma_start(
                        pwt_raw[i][:], pwT_d.ap()[128 * i : 128 * (i + 1), :]
                    )
                bias2_sb = const.tile([128, 3], F32, tag="bias2_sb")
                for oc in range(3):
                    psb = pwpsum.tile([128, 1], F32, tag="psb", name="psb")
                    for i in range(3):
                        # tiny free dims violate fp32r ISA restrictions; run
                        # these one-time matmuls as plain fp32 (bitcast)
                        nc.tensor.matmul(
                            psb[:],
                            pwt_raw[i][:, 128 * oc : 128 * (oc + 1)].bitcast(F32),
                            lnb_sb[:, i : i + 1].bitcast(F32),
                            start=(i == 0),
                            stop=(i == 2),
                        )
                    nc.scalar.activation(
                        bias2_sb[:, oc : oc + 1], psb[:], AF.Identity,
                        bias=pb_sb[:, oc : oc + 1],
                    )
                for i in range(3):
                    nc.vector.tensor_scalar_mul(
                        pwTs[i][:], pwt_raw[i][:], lnw_sb[:, i : i + 1]
                    )
                pwsumneg_row = const.tile([1, C], F32R, tag="pwsumneg_row")
                pssum = pwpsum.tile([1, C], F32, tag="pssum", name="pssum")
                for i in range(3):
                    nc.tensor.matmul(
                        pssum[:], ones_f[:], pwTs[i][:].bitcast(F32),
                        start=(i == 0), stop=(i == 2),
                    )
                nc.vector.tensor_scalar_mul(pwsumneg_row[:], pssum[:], -1.0 / C)

            # ---------------- resident big tensors ----------------
            x2T0 = big.tile([128, N], F16, tag="x2T0")     # x2T c 0:128, all N
            x2T1p = big.tile([128, NH], F16, tag="x2T1p")  # c 128:192; n-half j
            #                                              # on partitions 64j
            x1T0 = big.tile([128, NH], F16, tag="x1T0")    # x1T c 0:128, owned
            x1T1 = big.tile([64, NH], F16, tag="x1T1")     # x1T c 128:192, owned
            x2c0 = big.tile([128, M], F16, tag="x2c0")     # conv out, o 0:128
            x2c1 = big.tile([65, M], F16, tag="x2c1")      # o 128:192 + colsum
            # x2cT[m, mc, col]: cols 0:128 = o 0:128, 128:192 = o 128:192,
            # col 192 = colsum(x2c)[m]
            x2cT = big.tile([128, 8, 193], F16, tag="x2cT")
            # att tiles (lhsT for ch): cols 0:64 = att e 0:64, col 64 =
            # rowsum(att), cols 65:193 = att e 64:192 — the rowsum column
            # sits mid-tile so the Sigma row lands on psum partition 64
            # (engine reads must start on a partition-quarter boundary)
            att0s = big.tile([128, 193], F16, tag="att0s")  # d 0:128
            att1s = big.tile([64, 193], F16, tag="att1s")   # d 128:192

            # ---------------- phase 0 ----------------
            with tc.tile_pool(name="p0x", bufs=4) as p0x, \
                 tc.tile_pool(name="p0s", bufs=1, space="PSUM") as p0s, \
                 tc.tile_pool(name="p0tp", bufs=2, space="PSUM") as p0tp, \
                 tc.tile_pool(name="cvw", bufs=1) as cvw, \
                 tc.tile_pool(name="cvp", bufs=1, space="PSUM") as cvp:
                sT0 = p0s.tile([128, D], F32, tag="sT0", name="sT0")
                sT1 = p0s.tile([64, D], F32, tag="sT1", name="sT1")
                convw0 = cvw.tile([128, 16, 193], F16, tag="convw0", name="convw0")
                convw1 = cvw.tile([128, 16, 193], F16, tag="convw1", name="convw1")

                def load_convw():
                    nc.sync.dma_start(convw0[:], convw.ap()[0:128])
                    # convw1 (d_in 128:192) duplicated on both partition
                    # halves so its base partition matches the packed x2T1p
                    # slice it contracts against
                    nc.sync.dma_start(convw1[0:64], convw.ap()[128:192])
                    nc.sync.dma_start(convw1[64:128], convw.ap()[128:192])

                x2T0v = x2T0[:].rearrange(
                    "p (a i kh j kw) -> p a i kh j kw", a=2, i=16, kh=4, j=32, kw=4
                )
                x2T1v = x2T1p[:].rearrange(
                    "p (i kh j kw) -> p i kh j kw", i=16, kh=4, j=32, kw=4
                )

                def stream_quad(q):
                    xt = p0x.tile([128, 4, C], F16, tag="xt", name="xt")
                    nc.sync.dma_start(
                        xt[:],
                        x16.ap()[512 * q : 512 * (q + 1), :].rearrange(
                            "(t p) c -> p t c", p=128
                        ),
                    )
                    for j in range(4):
                        t = 4 * q + j
                        xtj = xt[:, j, :]
                        # sT[e, d] += x2_tile.T @ x1_tile (fp16, free 192)
                        nc.tensor.matmul(
                            sT0[:], xtj[:, 192:320], xtj[:, 0:192],
                            start=(t == 0), stop=(t == N // 128 - 1),
                        )
                        nc.tensor.matmul(
                            sT1[:], xtj[:, 320:384], xtj[:, 0:192],
                            start=(t == 0), stop=(t == N // 128 - 1),
                        )

                # xT piece loads, interleaved with the stream so conv inputs
                # arrive early while the s-matmul stream is still going
                def xt_piece(q):
                    if q < 8:  # x2T0 in 8 column pieces of 2048
                        nc.sync.dma_start(
                            x2T0[:, 2048 * q : 2048 * (q + 1)],
                            xt2.ap()[0:128, 2048 * q : 2048 * (q + 1)],
                        )
                    elif q < 12:  # x2T1p: half h on partitions 64h
                        j = q - 8
                        h, col = j // 2, 4096 * (j % 2)
                        nc.sync.dma_start(
                            x2T1p[64 * h : 64 * h + 64, col : col + 4096],
                            xt2.ap()[128:192, NH * h + col : NH * h + col + 4096],
                        )
                    elif q < 20:  # x1T0 in 8 pieces of 1024
                        j = q - 12
                        nc.sync.dma_start(
                            x1T0[:, 1024 * j : 1024 * (j + 1)],
                            xt1.ap()[0:128, 1024 * j : 1024 * (j + 1)],
                        )
                    elif q < 24:  # x1T1 in 4 pieces of 2048
                        j = q - 20
                        nc.sync.dma_start(
                            x1T1[:, 2048 * j : 2048 * (j + 1)],
                            xt1.ap()[128:192, 2048 * j : 2048 * (j + 1)],
                        )

                def conv_quarter(mq):
                    mh, ih = mq // 2, 8 * (mq % 2)
                    pc0 = cvp.tile([128, 256], F32, tag="pc0", name="pc0")
                    pc1 = cvp.tile([65, 256], F32, tag="pc1", name="pc1")
                    for khw in range(16):
                        kh, kw = khw // 4, khw % 4
                        rhs0 = x2T0v[:, mh, ih : ih + 8, kh, :, kw]
                        rhs1 = x2T1v[64 * mh : 64 * mh + 64, ih : ih + 8, kh, :, kw]
                        for (ps, osl) in ((pc0, slice(0, 128)), (pc1, slice(128, 193))):
                            nc.tensor.matmul(
                                ps[:], convw0[:, khw, osl], rhs0,
                                start=(khw == 0), stop=False,
                            )
                            nc.tensor.matmul(
                                ps[:],
                                convw1[64 * mh : 64 * mh + 64, khw, osl],
                                rhs1,
                                start=False, stop=(khw == 15),
                            )
                    mqq = 256 * mq
                    nc.scalar.activation(
                        x2c0[:, mqq : mqq + 256], pc0[:], AF.Identity,
                        bias=convb_sb[:, 0:1],
                    )
                    nc.scalar.activation(
                        x2c1[:, mqq : mqq + 256], pc1[:], AF.Identity,
                        bias=convb_sb[0:65, 1:2],
                    )

                for q in range(32):
                    stream_quad(q)
                    xt_piece(q)
                    if q == 2:
                        load_convw()
                    if q in (11, 14, 17, 20):
                        conv_quarter({11: 0, 14: 1, 17: 2, 20: 3}[q])

                # x2cT: transposes of x2c chunks; col 192 = colsum via the
                # conv's appended sum output channel (row 64 of x2c1).
                # Emitted before the softmax-dependent att transposes so the
                # PE isn't stalled on the softmax row ops.
                for mc in range(8):
                    tpc = p0tp.tile([128, 128], F16, tag="tpa", name="tpc")
                    nc.tensor.transpose(
                        tpc[:], x2c0[:, 128 * mc : 128 * (mc + 1)], ident16[:]
                    )
                    nc.vector.tensor_copy(x2cT[:, mc, 0:128], tpc[:])
                    tpd = p0tp.tile([128, 65], F16, tag="tpb", name="tpd")
                    nc.tensor.transpose(
                        tpd[:], x2c1[:, 128 * mc : 128 * (mc + 1)],
                        ident16[0:65, 0:65],
                    )
                    nc.vector.tensor_copy(x2cT[:, mc, 128:193], tpd[:])

                # ---------------- softmax over d (free axis of sT) ----------
                with tc.tile_pool(name="smx", bufs=1) as smx:
                    atts = {}
                    for (sps, ep, tagn) in ((sT0, 128, "attT0"), (sT1, 64, "attT1")):
                        mxn = smx.tile([ep, 1], F32, tag=f"mx{tagn}", name="mxn")
                        nc.vector.tensor_reduce(
                            mxn[:], sps[:ep, 0:D], mybir.AxisListType.X,
                            OP.max, negate=True,
                        )
                        expv = smx.tile([ep, D], F16, tag=f"ex{tagn}", name="expv")
                        nc.scalar.activation(
                            expv[:], sps[:ep, 0:D], AF.Exp, bias=mxn[:],
                        )
                        z = smx.tile([ep, 1], F32, tag=f"z{tagn}", name="z")
                        nc.vector.reduce_sum(z[:], expv[:], axis=mybir.AxisListType.X)
                        rz = smx.tile([ep, 1], F32, tag=f"rz{tagn}", name="rz")
                        nc.vector.reciprocal(rz[:], z[:])
                        att_t = smx.tile([ep, D], F16, tag=f"at{tagn}", name=tagn)
                        with nc.allow_low_precision(reason="fp16 att"):
                            nc.vector.tensor_scalar_mul(att_t[:], expv[:], rz[:])
                        atts[tagn] = att_t
                    attT0, attT1 = atts["attT0"], atts["attT1"]

                    # att = attT.T via 4 fp16 PE transposes; e 0:64 to cols
                    # 0:64, e 64:192 to cols 65:193 (col 64 = rowsum later)
                    tp1 = p0tp.tile([128, 128], F16, tag="tpa", name="tp1")
                    nc.tensor.transpose(tp1[:], attT0[:, 0:128], ident16[:])
                    nc.vector.tensor_copy(att0s[:, 0:64], tp1[:, 0:64])
                    nc.vector.tensor_copy(att0s[:, 65:129], tp1[:, 64:128])
                    tp2 = p0tp.tile([128, 64], F16, tag="tpb", name="tp2")
                    nc.tensor.transpose(tp2[:], attT1[:, 0:128], ident16[0:64, 0:64])
                    nc.vector.tensor_copy(att0s[:, 129:193], tp2[:])
                    tp3 = p0tp.tile([64, 128], F16, tag="tpa", name="tp3")
                    nc.tensor.transpose(tp3[:], attT0[:, 128:192], ident16[:])
                    nc.vector.tensor_copy(att1s[:, 0:64], tp3[:, 0:64])
                    nc.vector.tensor_copy(att1s[:, 65:129], tp3[:, 64:128])
                    tp4 = p0tp.tile([64, 64], F16, tag="tpb", name="tp4")
                    nc.tensor.transpose(tp4[:], attT1[:, 128:192], ident16[0:64, 0:64])
                    nc.vector.tensor_copy(att1s[:, 129:193], tp4[:])

                    # rowsum(att) -> col 64
                    with nc.allow_low_precision(reason="fp16 att rowsum"):
                        for ats, ep in ((att0s, 128), (att1s, 64)):
                            nc.vector.tensor_reduce(
                                ats[:, 64:65], ats[:, 0:64], mybir.AxisListType.X,
                                OP.add,
                            )
                            rtmp = smx.tile([ep, 1], F16, tag=f"rt{ep}",
                                            name="rtmp")
                            nc.vector.tensor_reduce(
                                rtmp[:], ats[:, 65:193], mybir.AxisListType.X,
                                OP.add,
                            )
                            nc.vector.tensor_tensor(
                                ats[:, 64:65], ats[:, 64:65], rtmp[:], OP.add
                            )

            # ---------------- main loop over n-blocks of the owned half -----
            with tc.tile_pool(name="mgt", bufs=2) as mgt, \
                 tc.tile_pool(name="mcat", bufs=5) as mcat, \
                 tc.tile_pool(name="msq", bufs=1) as msq, \
                 tc.tile_pool(name="mout", bufs=2) as mout, \
                 tc.tile_pool(name="mrows", bufs=2) as mrows, \
                 tc.tile_pool(name="mrows1", bufs=1) as mrows1, \
                 tc.tile_pool(name="pg", bufs=3, space="PSUM") as pg, \
                 tc.tile_pool(name="pwave", bufs=2, space="PSUM") as pwave, \
                 tc.tile_pool(name="pproj", bufs=2, space="PSUM") as pproj, \
                 tc.tile_pool(name="pmisc", bufs=1, space="PSUM") as pmisc:

                def emit_front(blk, qrows, qi):
                    nb = 512 * blk

                    gt = mgt.tile([128, 8, 512], F16, tag="gt", name="gt")
                    for mc in range(8):
                        psg = pg.tile([128, 512], F32, tag="pg", name="psg")
                        nc.tensor.matmul(
                            psg[:], x2c0[:, 128 * mc : 128 * (mc + 1)],
                            x1T0[:, nb : nb + 512],
                            start=True, stop=False,
                        )
                        nc.tensor.matmul(
                            psg[:], x2c1[0:64, 128 * mc : 128 * (mc + 1)],
                            x1T1[:, nb : nb + 512],
                            start=False, stop=True,
                        )
                        nc.scalar.activation(gt[:, mc, :], psg[:], AF.Sigmoid)

                    cat = mcat.tile([128, 3, 512], F32R, tag="cat", name="cat")
                    sq = msq.tile([128, 3, 512], F32R, tag="sq", name="sq")

                    # ch (+ rowsum row): cha rows = [e 0:64, Sig_ch]
                    ps_cha = pwave.tile([65, 512], F32, tag="cw", name="ps_cha")
                    nc.tensor.matmul(
                        ps_cha[:], att0s[:, 0:65], x2T0[:, nb : nb + 512],
                        start=True, stop=False,
                    )
                    nc.tensor.matmul(
                        ps_cha[:], att1s[:, 0:65], x2T1p[0:64, nb : nb + 512],
                        start=False, stop=True,
                    )
                    ps_chb = pwave.tile([128, 512], F32, tag="cw", name="ps_chb")
                    nc.tensor.matmul(
                        ps_chb[:], att0s[:, 65:193], x2T0[:, nb : nb + 512],
                        start=True, stop=False,
                    )
                    nc.tensor.matmul(
                        ps_chb[:], att1s[:, 65:193], x2T1p[0:64, nb : nb + 512],
                        start=False, stop=True,
                    )

                    # sp (+ colsum row): sp1 rows = [d 128:192, Sig_sp]
                    ps_sp0 = pwave.tile([128, 512], F32, tag="cw", name="ps_sp0")
                    for mc in range(8):
                        nc.tensor.matmul(
                            ps_sp0[:], x2cT[:, mc, 0:128], gt[:, mc, :],
                            start=(mc == 0), stop=(mc == 7),
                        )
                    ps_sp1 = pwave.tile([65, 512], F32, tag="cw", name="ps_sp1")
                    for mc in range(8):
                        nc.tensor.matmul(
                            ps_sp1[:], x2cT[:, mc, 128:193], gt[:, mc, :],
                            start=(mc == 0), stop=(mc == 7),
                        )

                    def evict(src, k, rows):
                        nc.any.tensor_copy(cat[rows, k, :], src)
                        nc.gpsimd.tensor_tensor(
                            sq[rows, k, :], cat[rows, k, :], cat[rows, k, :],
                            OP.mult,
                        )

                    evict(ps_sp0[:], 0, slice(0, 128))
                    evict(ps_sp1[0:64, :], 1, slice(0, 64))
                    evict(ps_cha[0:64, :], 1, slice(64, 128))
                    evict(ps_chb[:], 2, slice(0, 128))

                    # Sigma_cat row (raw sum over channels); only one PSUM
                    # operand allowed per op, so stage Sigma_sp via ACT
                    mu_r = qrows["mu"][:, qi, :]
                    spsum = mrows.tile([1, 512], F32, tag="rowS", name="spsum")
                    nc.scalar.activation(spsum[:], ps_sp1[64:65, :], AF.Identity)
                    nc.vector.tensor_tensor(
                        mu_r, ps_cha[64:65, :], spsum[:], OP.add
                    )

                    # E[cat^2] via ones-matmul on sq
                    ps_s2 = pproj.tile([1, 512], F32, tag="pp", name="ps_s2")
                    for k in range(3):
                        nc.tensor.matmul(
                            ps_s2[:], invC_col[:], sq[:, k, :],
                            start=(k == 0), stop=(k == 2),
                        )
                    # var = E[cat^2] - mu^2 (mu^2 staged in the var slice)
                    vsl = qrows["var"][:, qi, :]
                    nc.scalar.activation(vsl, mu_r, AF.Square, scale=1.0 / C)
                    nc.vector.tensor_tensor(vsl, ps_s2[:], vsl, OP.subtract)
                    return cat

                def emit_sqrt_quad(qrows):
                    # one Sqrt + one reciprocal over all 4 blocks' rows: keeps
                    # the ACT table swaps at <=2 per quad however the
                    # scheduler interleaves, since there is only one Sqrt op
                    nc.scalar.activation(
                        qrows["var"][:], qrows["var"][:], AF.Sqrt, bias=eps_sb[:]
                    )
                    with nc.allow_low_precision(reason="fp32r rstd, 4-byte"):
                        nc.vector.reciprocal(qrows["rstd"][:], qrows["var"][:])

                def emit_back(blk, cat, qrows, qi):
                    nb = 512 * blk
                    mu_r = qrows["mu"][:, qi, :]

                    ofin = mout.tile([128, 3, 512], F16, tag="ofin", name="ofin")

                    # proj matmuls depend only on cat/mu (ready since the
                    # front), so the first two are emitted before the
                    # rstd-dependent broadcast matmul to keep the PE busy
                    # while the ACT/DVE sqrt+reciprocal chain drains
                    def proj_oc(oc):
                        pso = pproj.tile([128, 512], F32, tag="pp", name="pso")
                        for k in range(3):
                            nc.tensor.matmul(
                                pso[:],
                                pwTs[k][:, 128 * oc : 128 * (oc + 1)],
                                cat[:, k, :],
                                start=(k == 0), stop=False,
                            )
                        nc.tensor.matmul(
                            pso[:],
                            pwsumneg_row[:, 128 * oc : 128 * (oc + 1)],
                            mu_r,
                            start=False, stop=True,
                        )
                        return pso

                    def osb_oc(oc, pso, rstd_bc):
                        osb = mout.tile([128, 512], F32, tag="osb", name="osb")
                        nc.vector.tensor_tensor(osb[:], pso[:], rstd_bc[:], OP.mult)
                        nc.gpsimd.tensor_scalar_add(
                            ofin[:, oc, :], osb[:], bias2_sb[:, oc : oc + 1]
                        )

                    pso0 = proj_oc(0)
                    pso1 = proj_oc(1)
                    ps_bc = pmisc.tile([128, 512], F32, tag="pm", name="ps_bc")
                    nc.tensor.matmul(
                        ps_bc[:], onesr_r[:], qrows["rstd"][:, qi, :],
                        start=True, stop=True,
                    )
                    rstd_bc = mout.tile([128, 512], F32, tag="rstd_bc",
                                        name="rstd_bc")
                    nc.any.tensor_copy(rstd_bc[:], ps_bc[:])
                    osb_oc(0, pso0, rstd_bc)
                    pso2 = proj_oc(2)
                    osb_oc(1, pso1, rstd_bc)
                    osb_oc(2, pso2, rstd_bc)
                    nc.sync.dma_start(
                        out_part.ap().rearrange(
                            "(oc p) n -> p oc n", p=128
                        )[:, :, nb : nb + 512],
                        ofin[:],
                    )

                for quad in range(NBLOCKS // 4):
                    blks = [4 * quad + i for i in range(4)]
                    qrows = {
                        "mu": mrows.tile([1, 4, 512], F32R, tag="rowB", name="mu_q"),
                        "var": mrows1.tile([1, 4, 512], F32, tag="rowD",
                                           name="var_q"),
                        "rstd": mrows.tile([1, 4, 512], F32R, tag="rowE",
                                           name="rstd_q"),
                    }
                    cats = [emit_front(b, qrows, i) for i, b in enumerate(blks)]
                    emit_sqrt_quad(qrows)
                    for i, (b, cat) in enumerate(zip(blks, cats)):
                        emit_back(b, cat, qrows, i)

    nc.finalize()
    return nc


_NC_CACHE: dict = {}


def _get_nc():
    if "nc" not in _NC_CACHE:
        _NC_CACHE["nc"] = build_nc()
    return _NC_CACHE["nc"]


def _prep_weights(conv_w, conv_b, ln_w, ln_b, proj_w, proj_b):
    conv_w = np.asarray(conv_w, dtype=np.float32)
    # [o, i, kh, kw] -> [i, khw, o] with appended sum-over-o column
    convw_t = np.ascontiguousarray(conv_w.transpose(1, 2, 3, 0)).reshape(D, 16, D)
    convw_ext = np.concatenate(
        [convw_t, convw_t.sum(axis=2, keepdims=True)], axis=2
    )  # [192, 16, 193]
    convb_ext = np.concatenate(
        [np.asarray(conv_b, dtype=np.float32),
         [np.asarray(conv_b, dtype=np.float32).sum()]]
    ).astype(np.float32)
    pwT = np.ascontiguousarray(np.asarray(proj_w, dtype=np.float32).T)
    return {
        "convw": convw_ext.astype(np.float16),
        "convb": convb_ext,
        "lnw": np.asarray(ln_w, dtype=np.float32),
        "lnb": np.asarray(ln_b, dtype=np.float32),
        "pwT": pwT,
        "pb": np.asarray(proj_b, dtype=np.float32),
    }


def _prep_in_map(x_b, weights):
    x16 = np.ascontiguousarray(x_b, dtype=np.float16)
    xT = np.ascontiguousarray(x16.T)  # [C, N]
    m = {
        "x16": x16,
        "xt1": np.ascontiguousarray(xT[0:D, 0:NH]),
        "xt2": np.ascontiguousarray(xT[D:C, :]),
    }
    m.update(weights)
    return m


def kernel(x, conv_w, conv_b, ln_w, ln_b, proj_w, proj_b, H=128, W=128):
    """Full-input entry point: shards over 8 cores (4 samples x 2 N-halves),
    runs the Bass kernel, gathers the full [B, C, N] output."""
    from concourse.bass_utils import run_bass_kernel_spmd

    x = np.asarray(x)
    assert x.shape == (B, N, C), x.shape

    weights = _prep_weights(conv_w, conv_b, ln_w, ln_b, proj_w, proj_b)
    nc = _get_nc()
    in_maps = []
    for b in range(B):
        for half in (0, 1):
            xb = x[b] if half == 0 else np.concatenate(
                [x[b, NH:], x[b, :NH]], axis=0
            )
            in_maps.append(_prep_in_map(xb, weights))
    res = run_bass_kernel_spmd(nc, in_maps, core_ids=list(range(8)))

    out = np.empty((B, C, N), dtype=np.float32)
    for b in range(B):
        for half in (0, 1):
            out[b][:, half * NH : (half + 1) * NH] = \
                res.results[2 * b + half]["out_part"].astype(np.float32)
    return out


# revision 23
# speedup vs baseline: 1.1129x; 1.1129x over previous
"""Trainium2 Bass kernel for nn_CSB (dense_transformer).

Reference computation (per sample b of B=4, N=16384, C=384, d=192, H=W=128,
M=N/16=1024):
  x1 = x[..., :d]; x2 = x[..., d:]
  x2c  = conv4x4s4(x2 as [d,H,W]) + conv_b            # [d, M]
  gate = sigmoid(x1 @ x2c)                            # [N, M]
  sp   = gate @ x2c.T                                 # [N, d]
  att  = softmax(x1.T @ x2, axis over first d)        # [d, d]
  ch   = x2 @ att                                     # [N, d]
  cat  = [sp, ch]; ln = LN(cat) * ln_w + ln_b
  out  = (ln @ proj_w.T + proj_b).T                   # [C, N]

Sharding: 8 cores = 4 samples x 2 N-halves. Each core gets the FULL sample
(halves swapped for half-1 cores so every core "owns" rows 0:8192) and
produces out[b][:, half*8192:(half+1)*8192].

Layout strategy (v2):
  - Host supplies, per core: x in fp16 natural layout (feeds only the
    s-matrix; fp16 matmuls run 1 cyc/row at any free size so free=192 needs
    no fp32r padding), plus PRE-TRANSPOSED fp16 x2T (full N, for conv/ch)
    and x1T (owned half, for the gate) — no on-chip PE transposes of x and
    no x1 re-load in the main loop.
  - conv weights are host-prepped [d_in, 16, 193] fp16 where output col 192
    is sum_o w[..,o]; the conv's pc1 psum row 64 then directly yields
    colsum(x2c)[m], which rides along the x2cT transposes into an appended
    lhsT column -> sum_d spT arrives as row 64 of the sp1 psum for free.
  - att tiles carry rowsum(att) as col 0, so sum_e chT arrives as row 0 of
    the cha psum. mean(cat) is then one DVE row-add; no stats matmuls.
  - LN folded into the projection:
      out = (pwTs.T @ catT + pwsumneg x Sigma) * rstd_bc + bias2
    with pwTs = proj_w.T * ln_w, pwsumneg = -colsum(pwTs)/C,
    bias2 = proj_w @ ln_b + proj_b.
  - Sqrt ops batched 4 blocks at a time so the ACT engine swaps between the
    sigmoid and sqrt table sets twice per 4 blocks instead of per block;
    squares and bias-adds run on the otherwise-idle GPSIMD engine.
  - Output written fp16 (within tolerance), host upcasts to f32.
"""

import sys
import types

_m = types.ModuleType("antenv.axon_hooks")
_m.get_axon_ntff_profile_hook = lambda: None
sys.modules.setdefault("antenv.axon_hooks", _m)

import numpy as np

import concourse.bacc as bacc
import concourse.mybir as mybir
import concourse.tile as tile
from concourse.masks import make_identity

F32 = mybir.dt.float32
F32R = mybir.dt.float32r
F16 = mybir.dt.float16
AF = mybir.ActivationFunctionType
OP = mybir.AluOpType

B = 4
N = 16384
C = 384
D = 192  # C // 2
M = 1024  # N // 16
NH = 8192  # N // 2, rows per core
NBLK = 512  # n-columns per main-loop block
NBLOCKS = NH // NBLK  # 16
EPS = 1e-5


def build_nc():
    nc = bacc.Bacc(None, target_bir_lowering=False)

    x16 = nc.dram_tensor("x16", [N, C], F16, kind="ExternalInput")
    xt1 = nc.dram_tensor("xt1", [D, NH], F16, kind="ExternalInput")
    xt2 = nc.dram_tensor("xt2", [D, N], F16, kind="ExternalInput")
    convw = nc.dram_tensor("convw", [D, 16, 193], F16, kind="ExternalInput")
    convb = nc.dram_tensor("convb", [193], F32, kind="ExternalInput")
    lnw_d = nc.dram_tensor("lnw", [C], F32, kind="ExternalInput")
    lnb_d = nc.dram_tensor("lnb", [C], F32R, kind="ExternalInput")
    pwT_d = nc.dram_tensor("pwT", [C, C], F32R, kind="ExternalInput")
    pb_d = nc.dram_tensor("pb", [C], F32, kind="ExternalInput")
    out_part = nc.dram_tensor("out_part", [C, NH], F16, kind="ExternalOutput")

    with tile.TileContext(nc) as tc:
        import contextlib

        with contextlib.ExitStack() as top:
            const = top.enter_context(tc.tile_pool(name="const", bufs=1))
            big = top.enter_context(tc.tile_pool(name="big", bufs=1))

            # ---------------- constants ----------------
            ident_f = const.tile([128, 128], F32, tag="ident_f")
            make_identity(nc, ident_f[:])
            ident16 = const.tile([128, 128], F16, tag="ident16")
            nc.vector.tensor_copy(ident16[:], ident_f[:])

            ones_f = const.tile([128, 1], F32, tag="ones_f")
            nc.gpsimd.memset(ones_f[:], 1.0)
            invC_f = const.tile([128, 1], F32, tag="invC_f")
            nc.gpsimd.memset(invC_f[:], 1.0 / C)
            invC_col = const.tile([128, 1], F32R, tag="invC_col")
            nc.vector.tensor_copy(invC_col[:], invC_f[:])
            onesr_f = const.tile([1, 128], F32, tag="onesr_f")
            nc.gpsimd.memset(onesr_f[:], 1.0)
            onesr_r = const.tile([1, 128], F32R, tag="onesr_r")
            nc.vector.tensor_copy(onesr_r[:], onesr_f[:])
            eps_sb = const.tile([1, 1], F32, tag="eps_sb")
            nc.gpsimd.memset(eps_sb[:], EPS)

            # per-channel vectors as [128, k] column stacks
            lnw_sb = const.tile([128, 3], F32, tag="lnw_sb")
            nc.sync.dma_start(lnw_sb[:], lnw_d.ap().rearrange("(o p) -> p o", p=128))
            lnb_sb = const.tile([128, 3], F32R, tag="lnb_sb")
            nc.sync.dma_start(lnb_sb[:], lnb_d.ap().rearrange("(o p) -> p o", p=128))
            pb_sb = const.tile([128, 3], F32, tag="pb_sb")
            nc.sync.dma_start(pb_sb[:], pb_d.ap().rearrange("(o p) -> p o", p=128))
            convb_sb = const.tile([128, 2], F32, tag="convb_sb")
            nc.sync.dma_start(convb_sb[:, 0:1], convb.ap()[0:128, None])
            nc.sync.dma_start(convb_sb[0:65, 1:2], convb.ap()[128:193, None])

            # proj weights: pwT [c, o]; pwTs = pwT * ln_w[c];
            # bias2 = P@lnb + pb; pwsumneg = -colsum(pwTs)/C
            pwTs = [
                const.tile([128, C], F32R, tag=f"pwTs{i}", name=f"pwTs{i}")
                for i in range(3)
            ]
            with tc.tile_pool(name="pwload", bufs=1) as pwload, \
                 tc.tile_pool(name="pwpsum", bufs=1, space="PSUM") as pwpsum:
                pwt_raw = [
                    pwload.tile([128, C], F32R, tag=f"pwt{i}", name=f"pwt{i}")
                    for i in range(3)
                ]
                for i in range(3):
                    nc.sync.dma_start(
                        pwt_raw[i][:], pwT_d.ap()[128 * i : 128 * (i + 1), :]
                    )
                bias2_sb = const.tile([128, 3], F32, tag="bias2_sb")
                for oc in range(3):
                    psb = pwpsum.tile([128, 1], F32, tag="psb", name="psb")
                    for i in range(3):
                        # tiny free dims violate fp32r ISA restrictions; run
                        # these one-time matmuls as plain fp32 (bitcast)
                        nc.tensor.matmul(
                            psb[:],
                            pwt_raw[i][:, 128 * oc : 128 * (oc + 1)].bitcast(F32),
                            lnb_sb[:, i : i + 1].bitcast(F32),
                            start=(i == 0),
                            stop=(i == 2),
                        )
                    nc.scalar.activation(
                        bias2_sb[:, oc : oc + 1], psb[:], AF.Identity,
                        bias=pb_sb[:, oc : oc + 1],
                    )
                for i in range(3):
                    nc.vector.tensor_scalar_mul(
                        pwTs[i][:], pwt_raw[i][:], lnw_sb[:, i : i + 1]
                    )
                pwsumneg_row = const.tile([1, C], F32R, tag="pwsumneg_row")
                pssum = pwpsum.tile([1, C], F32, tag="pssum", name="pssum")
                for i in range(3):
                    nc.tensor.matmul(
                        pssum[:], ones_f[:], pwTs[i][:].bitcast(F32),
                        start=(i == 0), stop=(i == 2),
                    )
                nc.vector.tensor_scalar_mul(pwsumneg_row[:], pssum[:], -1.0 / C)

            # ---------------- resident big tensors ----------------
            x2T0 = big.tile([128, N], F16, tag="x2T0")     # x2T c 0:128, all N
            x2T1p = big.tile([128, NH], F16, tag="x2T1p")  # c 128:192; n-half j
            #                                              # on partitions 64j
            x1T0 = big.tile([128, NH], F16, tag="x1T0")    # x1T c 0:128, owned
            x1T1 = big.tile([64, NH], F16, tag="x1T1")     # x1T c 128:192, owned
            x2c0 = big.tile([128, M], F16, tag="x2c0")     # conv out, o 0:128
            x2c1 = big.tile([65, M], F16, tag="x2c1")      # o 128:192 + colsum
            # x2cT[m, mc, col]: cols 0:128 = o 0:128, 128:192 = o 128:192,
            # col 192 = colsum(x2c)[m]
            x2cT = big.tile([128, 8, 193], F16, tag="x2cT")
            # att tiles (lhsT for ch): cols 0:64 = att e 0:64, col 64 =
            # rowsum(att), cols 65:193 = att e 64:192 — the rowsum column
            # sits mid-tile so the Sigma row lands on psum partition 64
            # (engine reads must start on a partition-quarter boundary)
            att0s = big.tile([128, 193], F16, tag="att0s")  # d 0:128
            att1s = big.tile([64, 193], F16, tag="att1s")   # d 128:192

            # ---------------- phase 0 ----------------
            with tc.tile_pool(name="p0x", bufs=4) as p0x, \
                 tc.tile_pool(name="p0s", bufs=1, space="PSUM") as p0s, \
                 tc.tile_pool(name="p0tp", bufs=2, space="PSUM") as p0tp, \
                 tc.tile_pool(name="cvw", bufs=1) as cvw, \
                 tc.tile_pool(name="cvp", bufs=1, space="PSUM") as cvp:
                sT0 = p0s.tile([128, D], F32, tag="sT0", name="sT0")
                sT1 = p0s.tile([64, D], F32, tag="sT1", name="sT1")
                convw0 = cvw.tile([128, 16, 193], F16, tag="convw0", name="convw0")
                convw1 = cvw.tile([128, 16, 193], F16, tag="convw1", name="convw1")

                def load_convw():
                    nc.sync.dma_start(convw0[:], convw.ap()[0:128])
                    # convw1 (d_in 128:192) duplicated on both partition
                    # halves so its base partition matches the packed x2T1p
                    # slice it contracts against
                    nc.sync.dma_start(convw1[0:64], convw.ap()[128:192])
                    nc.sync.dma_start(convw1[64:128], convw.ap()[128:192])

                x2T0v = x2T0[:].rearrange(
                    "p (a i kh j kw) -> p a i kh j kw", a=2, i=16, kh=4, j=32, kw=4
                )
                x2T1v = x2T1p[:].rearrange(
                    "p (i kh j kw) -> p i kh j kw", i=16, kh=4, j=32, kw=4
                )

                def stream_quad(q):
                    xt = p0x.tile([128, 4, C], F16, tag="xt", name="xt")
                    nc.sync.dma_start(
                        xt[:],
                        x16.ap()[512 * q : 512 * (q + 1), :].rearrange(
                            "(t p) c -> p t c", p=128
                        ),
                    )
                    for j in range(4):
                        t = 4 * q + j
                        xtj = xt[:, j, :]
                        # sT[e, d] += x2_tile.T @ x1_tile (fp16, free 192)
                        nc.tensor.matmul(
                            sT0[:], xtj[:, 192:320], xtj[:, 0:192],
                            start=(t == 0), stop=(t == N // 128 - 1),
                        )
                        nc.tensor.matmul(
                            sT1[:], xtj[:, 320:384], xtj[:, 0:192],
                            start=(t == 0), stop=(t == N // 128 - 1),
                        )

                # xT piece loads, interleaved with the stream so conv inputs
                # arrive early while the s-matmul stream is still going
                def xt_piece(q):
                    if q < 8:  # x2T0 in 8 column pieces of 2048
                        nc.sync.dma_start(
                            x2T0[:, 2048 * q : 2048 * (q + 1)],
                            xt2.ap()[0:128, 2048 * q : 2048 * (q + 1)],
                        )
                    elif q < 12:  # x2T1p: half h on partitions 64h
                        j = q - 8
                        h, col = j // 2, 4096 * (j % 2)
                        nc.sync.dma_start(
                            x2T1p[64 * h : 64 * h + 64, col : col + 4096],
                            xt2.ap()[128:192, NH * h + col : NH * h + col + 4096],
                        )
                    elif q < 20:  # x1T0 in 8 pieces of 1024
                        j = q - 12
                        nc.sync.dma_start(
                            x1T0[:, 1024 * j : 1024 * (j + 1)],
                            xt1.ap()[0:128, 1024 * j : 1024 * (j + 1)],
                        )
                    elif q < 24:  # x1T1 in 4 pieces of 2048
                        j = q - 20
                        nc.sync.dma_start(
                            x1T1[:, 2048 * j : 2048 * (j + 1)],
                            xt1.ap()[128:192, 2048 * j : 2048 * (j + 1)],
                        )

                def conv_quarter(mq):
                    mh, ih = mq // 2, 8 * (mq % 2)
                    pc0 = cvp.tile([128, 256], F32, tag="pc0", name="pc0")
                    pc1 = cvp.tile([65, 256], F32, tag="pc1", name="pc1")
                    for khw in range(16):
                        kh, kw = khw // 4, khw % 4
                        rhs0 = x2T0v[:, mh, ih : ih + 8, kh, :, kw]
                        rhs1 = x2T1v[64 * mh : 64 * mh + 64, ih : ih + 8, kh, :, kw]
                        for (ps, osl) in ((pc0, slice(0, 128)), (pc1, slice(128, 193))):
                            nc.tensor.matmul(
                                ps[:], convw0[:, khw, osl], rhs0,
                                start=(khw == 0), stop=False,
                            )
                            nc.tensor.matmul(
                                ps[:],
                                convw1[64 * mh : 64 * mh + 64, khw, osl],
                                rhs1,
                                start=False, stop=(khw == 15),
                            )
                    mqq = 256 * mq
                    nc.scalar.activation(
                        x2c0[:, mqq : mqq + 256], pc0[:], AF.Identity,
                        bias=convb_sb[:, 0:1],
                    )
                    nc.scalar.activation(
                        x2c1[:, mqq : mqq + 256], pc1[:], AF.Identity,
                        bias=convb_sb[0:65, 1:2],
                    )

                for q in range(32):
                    stream_quad(q)
                    xt_piece(q)
                    if q == 2:
                        load_convw()
                    if q in (11, 14, 17, 20):
                        conv_quarter({11: 0, 14: 1, 17: 2, 20: 3}[q])

                # x2cT: transposes of x2c chunks; col 192 = colsum via the
                # conv's appended sum output channel (row 64 of x2c1).
                # Emitted before the softmax-dependent att transposes so the
                # PE isn't stalled on the softmax row ops.
                for mc in range(8):
                    tpc = p0tp.tile([128, 128], F16, tag="tpa", name="tpc")
                    nc.tensor.transpose(
                        tpc[:], x2c0[:, 128 * mc : 128 * (mc + 1)], ident16[:]
                    )
                    nc.vector.tensor_copy(x2cT[:, mc, 0:128], tpc[:])
                    tpd = p0tp.tile([128, 65], F16, tag="tpb", name="tpd")
                    nc.tensor.transpose(
                        tpd[:], x2c1[:, 128 * mc : 128 * (mc + 1)],
                        ident16[0:65, 0:65],
                    )
                    nc.vector.tensor_copy(x2cT[:, mc, 128:193], tpd[:])

                # ---------------- softmax over d (free axis of sT) ----------
                with tc.tile_pool(name="smx", bufs=1) as smx:
                    atts = {}
                    for (sps, ep, tagn) in ((sT0, 128, "attT0"), (sT1, 64, "attT1")):
                        mxn = smx.tile([ep, 1], F32, tag=f"mx{tagn}", name="mxn")
                        nc.vector.tensor_reduce(
                            mxn[:], sps[:ep, 0:D], mybir.AxisListType.X,
                            OP.max, negate=True,
                        )
                        expv = smx.tile([ep, D], F16, tag=f"ex{tagn}", name="expv")
                        nc.scalar.activation(
                            expv[:], sps[:ep, 0:D], AF.Exp, bias=mxn[:],
                        )
                        z = smx.tile([ep, 1], F32, tag=f"z{tagn}", name="z")
                        nc.vector.reduce_sum(z[:], expv[:], axis=mybir.AxisListType.X)
                        rz = smx.tile([ep, 1], F32, tag=f"rz{tagn}", name="rz")
                        nc.vector.reciprocal(rz[:], z[:])
                        att_t = smx.tile([ep, D], F16, tag=f"at{tagn}", name=tagn)
                        with nc.allow_low_precision(reason="fp16 att"):
                            nc.vector.tensor_scalar_mul(att_t[:], expv[:], rz[:])
                        atts[tagn] = att_t
                    attT0, attT1 = atts["attT0"], atts["attT1"]

                    # att = attT.T via 4 fp16 PE transposes; e 0:64 to cols
                    # 0:64, e 64:192 to cols 65:193 (col 64 = rowsum later)
                    tp1 = p0tp.tile([128, 128], F16, tag="tpa", name="tp1")
                    nc.tensor.transpose(tp1[:], attT0[:, 0:128], ident16[:])
                    nc.vector.tensor_copy(att0s[:, 0:64], tp1[:, 0:64])
                    nc.vector.tensor_copy(att0s[:, 65:129], tp1[:, 64:128])
                    tp2 = p0tp.tile([128, 64], F16, tag="tpb", name="tp2")
                    nc.tensor.transpose(tp2[:], attT1[:, 0:128], ident16[0:64, 0:64])
                    nc.vector.tensor_copy(att0s[:, 129:193], tp2[:])
                    tp3 = p0tp.tile([64, 128], F16, tag="tpa", name="tp3")
                    nc.tensor.transpose(tp3[:], attT0[:, 128:192], ident16[:])
                    nc.vector.tensor_copy(att1s[:, 0:64], tp3[:, 0:64])
                    nc.vector.tensor_copy(att1s[:, 65:129], tp3[:, 64:128])
                    tp4 = p0tp.tile([64, 64], F16, tag="tpb", name="tp4")
                    nc.tensor.transpose(tp4[:], attT1[:, 128:192], ident16[0:64, 0:64])
                    nc.vector.tensor_copy(att1s[:, 129:193], tp4[:])

                    # rowsum(att) -> col 64
                    with nc.allow_low_precision(reason="fp16 att rowsum"):
                        for ats, ep in ((att0s, 128), (att1s, 64)):
                            nc.vector.tensor_reduce(
                                ats[:, 64:65], ats[:, 0:64], mybir.AxisListType.X,
                                OP.add,
                            )
                            rtmp = smx.tile([ep, 1], F16, tag=f"rt{ep}",
                                            name="rtmp")
                            nc.vector.tensor_reduce(
                                rtmp[:], ats[:, 65:193], mybir.AxisListType.X,
                                OP.add,
                            )
                            nc.vector.tensor_tensor(
                                ats[:, 64:65], ats[:, 64:65], rtmp[:], OP.add
                            )

            # ---------------- main loop over n-blocks of the owned half -----
            with tc.tile_pool(name="mgt", bufs=2) as mgt, \
                 tc.tile_pool(name="mcat", bufs=5) as mcat, \
                 tc.tile_pool(name="msq", bufs=1) as msq, \
                 tc.tile_pool(name="mout", bufs=2) as mout, \
                 tc.tile_pool(name="mrows", bufs=2) as mrows, \
                 tc.tile_pool(name="mrows1", bufs=1) as mrows1, \
                 tc.tile_pool(name="pg", bufs=3, space="PSUM") as pg, \
                 tc.tile_pool(name="pwave", bufs=2, space="PSUM") as pwave, \
                 tc.tile_pool(name="pproj", bufs=2, space="PSUM") as pproj, \
                 tc.tile_pool(name="pmisc", bufs=1, space="PSUM") as pmisc:

                def emit_front(blk, qrows, qi):
                    nb = 512 * blk

                    gt = mgt.tile([128, 8, 512], F16, tag="gt", name="gt")
                    for mc in range(8):
                        psg = pg.tile([128, 512], F32, tag="pg", name="psg")
                        nc.tensor.matmul(
                            psg[:], x2c0[:, 128 * mc : 128 * (mc + 1)],
                            x1T0[:, nb : nb + 512],
                            start=True, stop=False,
                        )
                        nc.tensor.matmul(
                            psg[:], x2c1[0:64, 128 * mc : 128 * (mc + 1)],
                            x1T1[:, nb : nb + 512],
                            start=False, stop=True,
                        )
                        nc.scalar.activation(gt[:, mc, :], psg[:], AF.Sigmoid)

                    cat = mcat.tile([128, 3, 512], F32R, tag="cat", name="cat")
                    sq = msq.tile([128, 3, 512], F32R, tag="sq", name="sq")

                    # ch (+ rowsum row): cha rows = [e 0:64, Sig_ch]
                    ps_cha = pwave.tile([65, 512], F32, tag="cw", name="ps_cha")
                    nc.tensor.matmul(
                        ps_cha[:], att0s[:, 0:65], x2T0[:, nb : nb + 512],
                        start=True, stop=False,
                    )
                    nc.tensor.matmul(
                        ps_cha[:], att1s[:, 0:65], x2T1p[0:64, nb : nb + 512],
                        start=False, stop=True,
                    )
                    ps_chb = pwave.tile([128, 512], F32, tag="cw", name="ps_chb")
                    nc.tensor.matmul(
                        ps_chb[:], att0s[:, 65:193], x2T0[:, nb : nb + 512],
                        start=True, stop=False,
                    )
                    nc.tensor.matmul(
                        ps_chb[:], att1s[:, 65:193], x2T1p[0:64, nb : nb + 512],
                        start=False, stop=True,
                    )

                    # sp (+ colsum row): sp1 rows = [d 128:192, Sig_sp]
                    ps_sp0 = pwave.tile([128, 512], F32, tag="cw", name="ps_sp0")
                    for mc in range(8):
                        nc.tensor.matmul(
                            ps_sp0[:], x2cT[:, mc, 0:128], gt[:, mc, :],
                            start=(mc == 0), stop=(mc == 7),
                        )
                    ps_sp1 = pwave.tile([65, 512], F32, tag="cw", name="ps_sp1")
                    for mc in range(8):
                        nc.tensor.matmul(
                            ps_sp1[:], x2cT[:, mc, 128:193], gt[:, mc, :],
                            start=(mc == 0), stop=(mc == 7),
                        )

                    def evict(src, k, rows):
                        nc.any.tensor_copy(cat[rows, k, :], src)
                        nc.any.tensor_tensor(
                            sq[rows, k, :], cat[rows, k, :], cat[rows, k, :],
                            OP.mult,
                        )

                    evict(ps_sp0[:], 0, slice(0, 128))
                    evict(ps_sp1[0:64, :], 1, slice(0, 64))
                    evict(ps_cha[0:64, :], 1, slice(64, 128))
                    evict(ps_chb[:], 2, slice(0, 128))

                    # Sigma_cat row (raw sum over channels); only one PSUM
                    # operand allowed per op, so stage Sigma_sp via ACT
                    mu_r = qrows["mu"][:, qi, :]
                    spsum = mrows.tile([1, 512], F32, tag="rowS", name="spsum")
                    nc.scalar.activation(spsum[:], ps_sp1[64:65, :], AF.Identity)
                    nc.vector.tensor_tensor(
                        mu_r, ps_cha[64:65, :], spsum[:], OP.add
                    )

                    # E[cat^2] via ones-matmul on sq
                    ps_s2 = pproj.tile([1, 512], F32, tag="pp", name="ps_s2")
                    for k in range(3):
                        nc.tensor.matmul(
                            ps_s2[:], invC_col[:], sq[:, k, :],
                            start=(k == 0), stop=(k == 2),
                        )
                    # var = E[cat^2] - mu^2 (mu^2 staged in the var slice)
                    vsl = qrows["var"][:, qi, :]
                    nc.scalar.activation(vsl, mu_r, AF.Square, scale=1.0 / C)
                    nc.vector.tensor_tensor(vsl, ps_s2[:], vsl, OP.subtract)
                    return cat

                def emit_sqrt_quad(qrows):
                    # one Sqrt + one reciprocal over all 4 blocks' rows: keeps
                    # the ACT table swaps at <=2 per quad however the
                    # scheduler interleaves, since there is only one Sqrt op
                    nc.scalar.activation(
                        qrows["var"][:], qrows["var"][:], AF.Sqrt, bias=eps_sb[:]
                    )
                    with nc.allow_low_precision(reason="fp32r rstd, 4-byte"):
                        nc.vector.reciprocal(qrows["rstd"][:], qrows["var"][:])

                def emit_back(blk, cat, qrows, qi):
                    nb = 512 * blk
                    mu_r = qrows["mu"][:, qi, :]

                    ofin = mout.tile([128, 3, 512], F16, tag="ofin", name="ofin")

                    # proj matmuls depend only on cat/mu (ready since the
                    # front), so the first two are emitted before the
                    # rstd-dependent broadcast matmul to keep the PE busy
                    # while the ACT/DVE sqrt+reciprocal chain drains
                    def proj_oc(oc):
                        pso = pproj.tile([128, 512], F32, tag="pp", name="pso")
                        for k in range(3):
                            nc.tensor.matmul(
                                pso[:],
                                pwTs[k][:, 128 * oc : 128 * (oc + 1)],
                                cat[:, k, :],
                                start=(k == 0), stop=False,
                            )
                        nc.tensor.matmul(
                            pso[:],
                            pwsumneg_row[:, 128 * oc : 128 * (oc + 1)],
                            mu_r,
                            start=False, stop=True,
                        )
                        return pso

                    def osb_oc(oc, pso, rstd_bc):
                        osb = mout.tile([128, 512], F32, tag="osb", name="osb")
                        nc.vector.tensor_tensor(osb[:], pso[:], rstd_bc[:], OP.mult)
                        nc.any.tensor_scalar_add(
                            ofin[:, oc, :], osb[:], bias2_sb[:, oc : oc + 1]
                        )

                    pso0 = proj_oc(0)
                    pso1 = proj_oc(1)
                    ps_bc = pmisc.tile([128, 512], F32, tag="pm", name="ps_bc")
                    nc.tensor.matmul(
                        ps_bc[:], onesr_r[:], qrows["rstd"][:, qi, :],
                        start=True, stop=True,
                    )
                    rstd_bc = mout.tile([128, 512], F32, tag="rstd_bc",
                                        name="rstd_bc")
                    nc.any.tensor_copy(rstd_bc[:], ps_bc[:])
                    osb_oc(0, pso0, rstd_bc)
                    pso2 = proj_oc(2)
                    osb_oc(1, pso1, rstd_bc)
                    osb_oc(2, pso2, rstd_bc)
                    nc.sync.dma_start(
                        out_part.ap().rearrange(
                            "(oc p) n -> p oc n", p=128
                        )[:, :, nb : nb + 512],
                        ofin[:],
                    )

                for quad in range(NBLOCKS // 4):
                    blks = [4 * quad + i for i in range(4)]
                    qrows = {
                        "mu": mrows.tile([1, 4, 512], F32R, tag="rowB", name="mu_q"),
                        "var": mrows1.tile([1, 4, 512], F32, tag="rowD",
                                           name="var_q"),
                        "rstd": mrows.tile([1, 4, 512], F32R, tag="rowE",
                                           name="rstd_q"),
                    }
                    cats = [emit_front(b, qrows, i) for i, b in enumerate(blks)]
                    emit_sqrt_quad(qrows)
                    for i, (b, cat) in enumerate(zip(blks, cats)):
                        emit_back(b, cat, qrows, i)

    nc.finalize()
    return nc


_NC_CACHE: dict = {}


def _get_nc():
    if "nc" not in _NC_CACHE:
        _NC_CACHE["nc"] = build_nc()
    return _NC_CACHE["nc"]


def _prep_weights(conv_w, conv_b, ln_w, ln_b, proj_w, proj_b):
    conv_w = np.asarray(conv_w, dtype=np.float32)
    # [o, i, kh, kw] -> [i, khw, o] with appended sum-over-o column
    convw_t = np.ascontiguousarray(conv_w.transpose(1, 2, 3, 0)).reshape(D, 16, D)
    convw_ext = np.concatenate(
        [convw_t, convw_t.sum(axis=2, keepdims=True)], axis=2
    )  # [192, 16, 193]
    convb_ext = np.concatenate(
        [np.asarray(conv_b, dtype=np.float32),
         [np.asarray(conv_b, dtype=np.float32).sum()]]
    ).astype(np.float32)
    pwT = np.ascontiguousarray(np.asarray(proj_w, dtype=np.float32).T)
    return {
        "convw": convw_ext.astype(np.float16),
        "convb": convb_ext,
        "lnw": np.asarray(ln_w, dtype=np.float32),
        "lnb": np.asarray(ln_b, dtype=np.float32),
        "pwT": pwT,
        "pb": np.asarray(proj_b, dtype=np.float32),
    }


def _prep_in_map(x_b, weights):
    x16 = np.ascontiguousarray(x_b, dtype=np.float16)
    xT = np.ascontiguousarray(x16.T)  # [C, N]
    m = {
        "x16": x16,
        "xt1": np.ascontiguousarray(xT[0:D, 0:NH]),
        "xt2": np.ascontiguousarray(xT[D:C, :]),
    }
    m.update(weights)
    return m


def kernel(x, conv_w, conv_b, ln_w, ln_b, proj_w, proj_b, H=128, W=128):
    """Full-input entry point: shards over 8 cores (4 samples x 2 N-halves),
    runs the Bass kernel, gathers the full [B, C, N] output."""
    from concourse.bass_utils import run_bass_kernel_spmd

    x = np.asarray(x)
    assert x.shape == (B, N, C), x.shape

    weights = _prep_weights(conv_w, conv_b, ln_w, ln_b, proj_w, proj_b)
    nc = _get_nc()
    in_maps = []
    for b in range(B):
        for half in (0, 1):
            xb = x[b] if half == 0 else np.concatenate(
                [x[b, NH:], x[b, :NH]], axis=0
            )
            in_maps.append(_prep_in_map(xb, weights))
    res = run_bass_kernel_spmd(nc, in_maps, core_ids=list(range(8)))

    out = np.empty((B, C, N), dtype=np.float32)
    for b in range(B):
        for half in (0, 1):
            out[b][:, half * NH : (half + 1) * NH] = \
                res.results[2 * b + half]["out_part"].astype(np.float32)
    return out
